# revision 1
# baseline (speedup 1.0000x reference)
"""Trainium2 Bass kernel for AttnAdaINCos (cosine-attention AdaIN style transfer).

Sharding: 8 cores = 4 batches x 2 content-pixel halves (data-parallel batch,
sequence-parallel content tokens; style tokens replicated per core).

Per-core math (batch b, local pixels p in one half, style tokens j):
  F = Wf ck + bf              [C, NL]   (content-key features, channel-major)
  G = Wg sk + bg              [C, N]
  Hs = Wh st + bh  -> HsT     [N, C]    (computed directly transposed)
  f_p = ||F_:p|| + eps, g_j = ||G_:j|| + eps
  t'_jp = (G^T F)_jp + g_j f_p          (rank-1 term fused as K=C+1 matmul)
  D_jp = relu(t'_jp) / g_j              (PSUM evac: relu with per-partition scale)
  r_p = sum_j D_jp, den_p = r_p + eps * f_p
  mean[c,p] = (HsT^T D)[c,p] / den_p
  sq[c,p]   = ((HsT^2)^T D)[c,p] / den_p
  std = sqrt(relu(sq - mean^2))
  out = std * (ct - cmean_c)/cstd_c + mean     (cmean/cstd over full batch pixels)

The /g_j scale makes D = f~ * A where A is the reference's relu'd cosine
affinity; the f~ factor cancels exactly in mean/sq via den.

Dtypes: all five matrix products (3 convs + 2 attention stages) run fp8e4
with DoubleRow (2 contraction blocks per matmul) on host-interleaved
[128, K/256, 2, n] operands, accumulating fp32 in PSUM; the rank-1
augmentation and column-norm reductions run bf16; content stats and the
final AdaIN combine run fp32. Measured 345.7us / rel err 4.9e-3 on HW.
"""

import sys

if "/opt/trn_rl_repo" not in sys.path:
    sys.path.insert(0, "/opt/trn_rl_repo")

from contextlib import ExitStack

import numpy as np

import ml_dtypes

import concourse.bass as bass
import concourse.tile as tile
from concourse import bacc, mybir
from concourse.bass_utils import run_bass_kernel_spmd

F32 = mybir.dt.float32
F32R = mybir.dt.float32r
BF16 = mybir.dt.bfloat16
FP8 = mybir.dt.float8e4
DR = mybir.MatmulPerfMode.DoubleRow
AF = mybir.ActivationFunctionType
ALU = mybir.AluOpType
PS = bass.MemorySpace.PSUM
EPS = 1e-5
NC = 512  # free-dim chunk size (one PSUM bank of fp32)


def build_nc(C=512, N=4096, NL=2048):
    """Build the single SPMD program (identical on all cores)."""
    KB = C // 128     # channel 128-blocks (contraction blocks)
    CB = C // 128     # output-channel 128-blocks
    QC = N // NC      # style-token 512-chunks
    QB = N // 128     # style-token 128-blocks (j-blocks)
    PC = NL // NC     # local-pixel 512-chunks
    NT = 2 * NL       # full-batch pixel count (for content stats)

    nc = bacc.Bacc("TRN2", target_bir_lowering=False)

    K2 = KB // 2
    ck = nc.dram_tensor("ck", [128, K2, 2, NL], FP8, kind="ExternalInput")
    sk = nc.dram_tensor("sk", [128, K2, 2, N], FP8, kind="ExternalInput")
    st = nc.dram_tensor("st", [128, K2, 2, N], FP8, kind="ExternalInput")
    ct = nc.dram_tensor("ct", [C, NT], F32, kind="ExternalInput")
    wf = nc.dram_tensor("wf", [128, K2, 2, C], FP8, kind="ExternalInput")
    wg = nc.dram_tensor("wg", [128, K2, 2, C], FP8, kind="ExternalInput")
    wh = nc.dram_tensor("wh", [128, K2, 2, C], FP8, kind="ExternalInput")
    bfb = nc.dram_tensor("bfb", [128, KB], F32, kind="ExternalInput")
    bgb = nc.dram_tensor("bgb", [128, KB], F32, kind="ExternalInput")
    bhb = nc.dram_tensor("bhb", [1, C], F32, kind="ExternalInput")
    out = nc.dram_tensor("out", [C, NL], F32, kind="ExternalOutput")

    with tile.TileContext(nc) as tc:
        with ExitStack() as stk:
            const = stk.enter_context(tc.tile_pool(name="const", bufs=1))
            fpool = stk.enter_context(tc.tile_pool(name="fpool", bufs=1))
            gpool = stk.enter_context(tc.tile_pool(name="gpool", bufs=1))
            hpool = stk.enter_context(tc.tile_pool(name="hpool", bufs=1))
            pp = stk.enter_context(tc.tile_pool(name="pp", bufs=8, space=PS))
            small = stk.enter_context(tc.tile_pool(name="small", bufs=4))
            drp = stk.enter_context(
                tc.tile_pool(name="drp", bufs=1, space=bass.MemorySpace.DRAM))
            stg = stk.enter_context(tc.tile_pool(name="stg", bufs=16))
            etmp = stk.enter_context(tc.tile_pool(name="etmp", bufs=6))
            dp = stk.enter_context(tc.tile_pool(name="dp", bufs=1))
            cmb = stk.enter_context(tc.tile_pool(name="cmb", bufs=3))
            ctl = stk.enter_context(tc.tile_pool(name="ctl", bufs=6))
            op = stk.enter_context(tc.tile_pool(name="op", bufs=4))
            ivd = stk.enter_context(tc.tile_pool(name="ivd", bufs=2))

            # ---- persistent tiles ----
            wfS = const.tile([128, K2, 2, C], FP8, tag="wf", name="wf_s")
            wgS = const.tile([128, K2, 2, C], FP8, tag="wg", name="wg_s")
            whS = const.tile([128, K2, 2, C], FP8, tag="wh", name="wh_s")
            bf_sb = const.tile([128, KB], F32, tag="bf", name="bf")
            bg_sb = const.tile([128, KB], F32, tag="bg", name="bg")
            bh_bc = const.tile([128, C], F32, tag="bh", name="bh")
            ones_bf = const.tile([128, 1], BF16, tag="ones", name="ones")
            g_row = const.tile([1, N], BF16, tag="grow", name="grow")
            f_row = const.tile([1, NL], BF16, tag="frow", name="frow")
            gT_bf = const.tile([128, QB], BF16, tag="gT", name="gT")
            invgT = const.tile([128, QB], F32, tag="invgT", name="invgT")
            cmean = const.tile([128, CB], F32, tag="cmean", name="cmean")
            cinv = const.tile([128, CB], F32, tag="cinv", name="cinv")

            # fp8 operand tiles with DoubleRow 2-plane interleave: plane i of a
            # [128, 2, n] tile holds channel/token block (2*k2 + i)
            HP = ((2 * C + 1 + 15) // 16) * 16  # plane stride %16==0 for DoubleRow
            F2 = [fpool.tile([128, 2, NL], FP8, tag=f"F{k2}", name=f"F{k2}")
                  for k2 in range(KB // 2)]
            G2 = [gpool.tile([128, 2, N], FP8, tag=f"G{k2}", name=f"G{k2}")
                  for k2 in range(KB // 2)]
            # [Hs | Hs^2 | 1 | pad] per j-block pair: stage-2 stationary operand
            H2 = [hpool.tile([128, 2, HP], FP8, tag=f"H{j2}", name=f"H{j2}")
                  for j2 in range(QB // 2)]

            eps_sb = const.tile([128, 1], F32, tag="eps", name="eps")
            nc.vector.memset(eps_sb, EPS)
            nc.vector.memset(ones_bf, 1.0)
            nc.sync.dma_start(out=bf_sb, in_=bfb[:, :])
            nc.sync.dma_start(out=bg_sb, in_=bgb[:, :])
            bh_ap = bhb[0:1, :]
            nc.sync.dma_start(
                out=bh_bc,
                in_=bass.AP(tensor=bh_ap.tensor, offset=bh_ap.offset,
                            ap=[[0, 128], [1, C]]),
            )

            def load_weight(w_dram, w_t):
                for kb in range(KB):
                    nc.sync.dma_start(out=w_t[:, kb, :],
                                      in_=w_dram[kb * 128:(kb + 1) * 128, :])

            # ---- F = Wf ck + bf (channel-major [C, NL]) + column norms f ----
            nc.sync.dma_start(out=wfS, in_=wf[:, :, :, :])
            for pc in range(PC):
                ckc = []
                for k2 in range(K2):
                    s = stg.tile([128, 2, NC], FP8, tag="stg", name="stg")
                    nc.sync.dma_start(
                        out=s, in_=ck[:, k2, :, pc * NC:(pc + 1) * NC])
                    ckc.append(s)
                f2ps = pp.tile([1, NC], F32, tag="ps", name="ps")
                for ob in range(CB):
                    ps = pp.tile([128, NC], F32, tag="ps", name="ps")
                    for k2 in range(K2):
                        nc.tensor.matmul(ps,
                                         wfS[:, k2, :, ob * 128:(ob + 1) * 128],
                                         ckc[k2], perf_mode=DR, start=(k2 == 0),
                                         stop=(k2 == K2 - 1))
                    nc.scalar.activation(
                        out=F2[ob // 2][:, ob % 2, pc * NC:(pc + 1) * NC],
                        in_=ps, func=AF.Identity, bias=bf_sb[:, ob:ob + 1])
                    fsq = etmp.tile([128, NC], BF16, tag="esq", name="esq")
                    fpl = F2[ob // 2][:, ob % 2, pc * NC:(pc + 1) * NC]
                    nc.vector.tensor_mul(fsq, fpl, fpl)
                    nc.tensor.matmul(f2ps, ones_bf, fsq, start=(ob == 0),
                                     stop=(ob == CB - 1))
                nc.scalar.activation(out=f_row[0:1, pc * NC:(pc + 1) * NC],
                                     in_=f2ps, func=AF.Sqrt)
                nc.vector.tensor_scalar_add(f_row[0:1, pc * NC:(pc + 1) * NC],
                                            f_row[0:1, pc * NC:(pc + 1) * NC], EPS)

            # ---- G = Wg sk + bg + column norms g (gT/invgT per-partition) ----
            nc.sync.dma_start(out=wgS, in_=wg[:, :, :, :])
            gd = drp.tile([1, N], BF16, tag="gd", name="gd")
            for qc in range(QC):
                skc = []
                for k2 in range(K2):
                    s = stg.tile([128, 2, NC], FP8, tag="stg", name="stg")
                    nc.sync.dma_start(
                        out=s, in_=sk[:, k2, :, qc * NC:(qc + 1) * NC])
                    skc.append(s)
                g2ps = pp.tile([1, NC], F32, tag="ps", name="ps")
                for ob in range(CB):
                    ps = pp.tile([128, NC], F32, tag="ps", name="ps")
                    for k2 in range(K2):
                        nc.tensor.matmul(ps,
                                         wgS[:, k2, :, ob * 128:(ob + 1) * 128],
                                         skc[k2], perf_mode=DR, start=(k2 == 0),
                                         stop=(k2 == K2 - 1))
                    nc.scalar.activation(
                        out=G2[ob // 2][:, ob % 2, qc * NC:(qc + 1) * NC],
                        in_=ps, func=AF.Identity, bias=bg_sb[:, ob:ob + 1])
                    gsq = etmp.tile([128, NC], BF16, tag="esq", name="esq")
                    gpl = G2[ob // 2][:, ob % 2, qc * NC:(qc + 1) * NC]
                    nc.vector.tensor_mul(gsq, gpl, gpl)
                    nc.tensor.matmul(g2ps, ones_bf, gsq, start=(ob == 0),
                                     stop=(ob == CB - 1))
                nc.scalar.activation(out=g_row[0:1, qc * NC:(qc + 1) * NC],
                                     in_=g2ps, func=AF.Sqrt)
                nc.vector.tensor_scalar_add(g_row[0:1, qc * NC:(qc + 1) * NC],
                                            g_row[0:1, qc * NC:(qc + 1) * NC], EPS)
                # bounce row chunk to DRAM for the [1,N]->[128,QB] scatter
                nc.sync.dma_start(out=gd[0:1, qc * NC:(qc + 1) * NC],
                                  in_=g_row[0:1, qc * NC:(qc + 1) * NC])
            nc.sync.dma_start(
                out=gT_bf,
                in_=gd.rearrange("p (c r) -> (p r) c", r=128))
            gT_f = small.tile([128, QB], F32, tag="gTf", name="gTf")
            nc.vector.tensor_copy(out=gT_f, in_=gT_bf)
            nc.vector.reciprocal(out=invgT, in_=gT_f)

            # ---- HsT[j, c] = st^T WhT + bh ; H2 = [Hs | Hs^2 | 1] ----
            nc.sync.dma_start(out=whS, in_=wh[:, :, :, :])
            for qc in range(QC):
                stc = []
                for k2 in range(K2):
                    s = stg.tile([128, 2, NC], FP8, tag="stg", name="stg")
                    nc.sync.dma_start(
                        out=s, in_=st[:, k2, :, qc * NC:(qc + 1) * NC])
                    stc.append(s)
                for mi in range(NC // 128):
                    jb = qc * (NC // 128) + mi
                    ps = pp.tile([128, C], F32, tag="ps", name="ps")
                    for k2 in range(K2):
                        nc.tensor.matmul(ps,
                                         stc[k2][:, :, mi * 128:(mi + 1) * 128],
                                         whS[:, k2, :, :], perf_mode=DR,
                                         start=(k2 == 0), stop=(k2 == K2 - 1))
                    hpl = H2[jb // 2][:, jb % 2, :]
                    nc.vector.tensor_add(hpl[:, 0:C], ps, bh_bc)
                    nc.scalar.activation(out=hpl[:, C:2 * C],
                                         in_=hpl[:, 0:C], func=AF.Square)
                    nc.gpsimd.memset(hpl[:, 2 * C:2 * C + 1], 1.0)

            # ---- main loop ----
            for pc in range(PC):
                psl = slice(pc * NC, (pc + 1) * NC)
                # stage 1: D_jp = relu(G^T F + g f) / g
                D = []
                for qb in range(QB):
                    ps1 = pp.tile([128, NC], F32, tag="ps", name="ps")
                    for k2 in range(KB // 2):
                        nc.tensor.matmul(ps1,
                                         G2[k2][:, :, qb * 128:(qb + 1) * 128],
                                         F2[k2][:, :, psl], perf_mode=DR,
                                         start=(k2 == 0), stop=False)
                    nc.tensor.matmul(ps1, g_row[0:1, qb * 128:(qb + 1) * 128],
                                     f_row[0:1, psl], start=False, stop=True)
                    if qb % 2 == 0:
                        d2 = dp.tile([128, 2, NC], FP8, tag=f"d{qb // 2}",
                                     name=f"d{qb // 2}")
                        D.append(d2)
                    dpl = D[qb // 2][:, qb % 2, :]
                    if qb % 4 != 3:
                        nc.scalar.activation(out=dpl, in_=ps1, func=AF.Relu,
                                             scale=invgT[:, qb:qb + 1])
                    else:
                        nc.vector.tensor_scalar(dpl, ps1, invgT[:, qb:qb + 1],
                                                0.0, ALU.mult, ALU.max)

                if pc == 0:
                    # content stats, deferred here so their DMA + DVE work
                    # overlaps stage-2 matmuls instead of the input-load crunch
                    nsub = NT // NC
                    for cb in range(CB):
                        stats = small.tile([128, nsub, nc.vector.BN_STATS_DIM],
                                           F32, tag="bnstats", name="bnstats")
                        for s_i in range(nsub):
                            s = ctl.tile([128, NC], F32, tag="ctl", name="ctl")
                            nc.sync.dma_start(
                                out=s, in_=ct[cb * 128:(cb + 1) * 128,
                                              s_i * NC:(s_i + 1) * NC])
                            nc.vector.bn_stats(out=stats[:, s_i, :], in_=s)
                        mv = small.tile([128, nc.vector.BN_AGGR_DIM], F32,
                                        tag="bnmv", name="bnmv")
                        nc.vector.bn_aggr(out=mv, in_=stats)
                        nc.gpsimd.tensor_copy(out=cmean[:, cb:cb + 1],
                                              in_=mv[:, 0:1])
                        cstd = small.tile([128, 1], F32, tag="cstd", name="cstd")
                        nc.scalar.activation(out=cstd, in_=mv[:, 1:2],
                                             func=AF.Sqrt, bias=eps_sb,
                                             scale=float(NT) / (NT - 1))
                        nc.vector.reciprocal(out=cinv[:, cb:cb + 1], in_=cstd)

                # stage 2a: row sums (ones block) -> den -> invden broadcast
                psr = pp.tile([1, NC], F32, tag="ps", name="ps")
                for j2 in range(QB // 2):
                    nc.tensor.matmul(psr, H2[j2][:, :, 2 * C:2 * C + 1], D[j2],
                                     perf_mode=DR, start=(j2 == 0),
                                     stop=(j2 == QB // 2 - 1))
                den = ivd.tile([1, NC], F32, tag="den", name="den")
                nc.vector.scalar_tensor_tensor(den, f_row[0:1, psl], EPS, psr,
                                               op0=ALU.mult, op1=ALU.add)
                nc.vector.reciprocal(den, den)
                ivbc = ivd.tile([128, NC], F32, tag="ivbc", name="ivbc")
                nc.gpsimd.partition_broadcast(ivbc, den)

                # stage 2b: mean/sq numerators + combine per channel block
                for cb in range(CB):
                    psm = pp.tile([128, NC], F32, tag="ps", name="ps")
                    for j2 in range(QB // 2):
                        nc.tensor.matmul(psm,
                                         H2[j2][:, :, cb * 128:(cb + 1) * 128],
                                         D[j2], perf_mode=DR, start=(j2 == 0),
                                         stop=(j2 == QB // 2 - 1))
                    pss = pp.tile([128, NC], F32, tag="ps", name="ps")
                    for j2 in range(QB // 2):
                        nc.tensor.matmul(pss,
                                         H2[j2][:, :,
                                                C + cb * 128:C + (cb + 1) * 128],
                                         D[j2], perf_mode=DR, start=(j2 == 0),
                                         stop=(j2 == QB // 2 - 1))
                    ctt = ctl.tile([128, NC], F32, tag="ctl", name="ctl")
                    nc.sync.dma_start(out=ctt,
                                      in_=ct[cb * 128:(cb + 1) * 128, psl])
                    mean_t = cmb.tile([128, NC], F32, tag="mean", name="mean")
                    nc.vector.tensor_mul(mean_t, psm, ivbc)
                    sqs_t = cmb.tile([128, NC], F32, tag="sqs", name="sqs")
                    nc.vector.tensor_mul(sqs_t, pss, ivbc)
                    m2_t = cmb.tile([128, NC], F32, tag="m2", name="m2")
                    nc.scalar.activation(out=m2_t, in_=mean_t, func=AF.Square)
                    nc.vector.scalar_tensor_tensor(sqs_t, m2_t, -1.0, sqs_t,
                                                   op0=ALU.mult, op1=ALU.add)
                    nc.vector.tensor_scalar_max(sqs_t, sqs_t, 0.0)
                    nc.scalar.activation(out=m2_t, in_=sqs_t, func=AF.Sqrt)
                    out_t = op.tile([128, NC], F32, tag="out", name="out_t")
                    nc.vector.tensor_scalar(out_t, ctt, cmean[:, cb:cb + 1],
                                            cinv[:, cb:cb + 1], ALU.subtract,
                                            ALU.mult)
                    nc.vector.tensor_mul(out_t, out_t, m2_t)
                    nc.vector.tensor_add(out_t, out_t, mean_t)
                    nc.sync.dma_start(out=out[cb * 128:(cb + 1) * 128, psl],
                                      in_=out_t)

    nc.finalize()
    return nc


_NC_CACHE = {}


def _get_nc(C, N, NL):
    key = (C, N, NL)
    if key not in _NC_CACHE:
        _NC_CACHE[key] = build_nc(C, N, NL)
    return _NC_CACHE[key]


def make_in_maps(content, style, content_key, style_key, Wf, bf, Wg, bg, Wh, bh):
    """Shard full inputs into 8 per-core input maps."""
    B, C, H, W = content.shape
    NP = H * W
    NL = NP // 2
    KB = C // 128

    def prep(x):
        return np.ascontiguousarray(x, dtype=np.float32)

    def prep16(x):
        return np.ascontiguousarray(x).astype(ml_dtypes.bfloat16)

    def prep8i(x):  # [C, n] -> [128, KB//2, 2, n] fp8 DoubleRow interleave
        Cd, n = x.shape
        k2 = Cd // 256
        return np.ascontiguousarray(
            np.asarray(x).reshape(k2, 2, 128, n).transpose(2, 0, 1, 3)
        ).astype(ml_dtypes.float8_e4m3)

    wfT = prep8i(np.asarray(Wf).T)
    wgT = prep8i(np.asarray(Wg).T)
    whT = prep8i(np.asarray(Wh).T)
    bfb = prep(np.asarray(bf).reshape(KB, 128).T)
    bgb = prep(np.asarray(bg).reshape(KB, 128).T)
    bhb = prep(np.asarray(bh).reshape(1, C))

    in_maps = []
    for core in range(8):
        b, h = core // 2, core % 2
        ctf = np.asarray(content[b]).reshape(C, NP)
        if h == 1:  # local half first (stats are permutation-invariant)
            ctf = np.concatenate([ctf[:, NL:], ctf[:, :NL]], axis=1)
        in_maps.append({
            "ck": prep8i(np.asarray(content_key[b]).reshape(C, NP)[:, h * NL:(h + 1) * NL]),
            "sk": prep8i(np.asarray(style_key[b]).reshape(C, NP)),
            "st": prep8i(np.asarray(style[b]).reshape(C, NP)),
            "ct": prep(ctf),
            "wf": wfT, "wg": wgT, "wh": whT,
            "bfb": bfb, "bgb": bgb, "bhb": bhb,
        })
    return in_maps


def kernel(content, style, content_key, style_key, Wf, bf, Wg, bg, Wh, bh,
           _trace=False):
    B, C, H, W = content.shape
    NP = H * W
    NL = NP // 2
    nc = _get_nc(C, NP, NL)
    in_maps = make_in_maps(content, style, content_key, style_key,
                           Wf, bf, Wg, bg, Wh, bh)
    res = run_bass_kernel_spmd(nc, in_maps, core_ids=list(range(8)), trace=_trace)
    out = np.empty((B, C, NP), dtype=np.float32)
    for core in range(8):
        b, h = core // 2, core % 2
        out[b, :, h * NL:(h + 1) * NL] = res.results[core]["out"]
    if _trace:
        kernel.last_results = res
    return out.reshape(B, C, H, W)

